# revision 26
# baseline (speedup 1.0000x reference)
"""CKANKANNet Trainium2 kernel (per-core SPMD program, B=8 samples/core).

Basis algorithm (d3-direct, weight-folded): with v = 2.5x+5.5 mapped to
integer knots and V = min(v, 11) via RY = relu(5.5-2.5x), the 3rd finite
difference of relu cubes saturates: d3_m(t) = t^3 - 3 relu(t-1)^3
+ 3 relu(t-2)^3 for t = clamp(V-m, 0, 3), constant 6 beyond. Each d3 slab
is THREE fused single-uop DVE ops straight from RY (clamps commuted into
the op bodies: u=max(min(RY+m-11,0),-3), m1=max(min(RY+m-10,0),-2),
m2=max(min(RY+m-9,0),-1); P1: 3*m1^3; P2: A - u^3; P3: A2 - 3*m2^3).
Since conv/linear are linear in the basis, the d4_j = d3_j - d3_{j+1}
difference is folded into the weights (w'_m = w_m - w_{m-1}, 9 channels):
no subtract passes, and L3/linear consume D3 tiles directly. The /6 is
folded into the weights too.

Convs: fp16 matmuls, channels on K partitions, 3x3 taps as accumulating
matmuls with edge-trimmed N ranges. L1 bakes ky taps into K via 3 shifted
channel-block copies (K=96, 30 live rows each). L2 K = 5 full 128-blocks
(4 m-pairs + [base|m8]). L3 K = 10 blocks (9 m + base). Image halves are
{0,1,4,5}/{2,3,6,7} at every layer so each layer's half-0 compute overlaps
the previous layer's half-1. Linear uses m-pair packed K=128 stationaries
(pr4 = [m8|base]).
"""
import sys
sys.path.insert(0, '/opt/trn_rl_repo')
from contextlib import ExitStack

import numpy as np
MM_NP = np.float16

import concourse.bass as bass
import concourse.tile as tile
from concourse import bacc, mybir
from concourse import dve_ops
from concourse.dve_spec import (Spec, Src0, Src1, sq, lower, minn, maxx,
                                _has_src1, C0, C1, C2, Zero, One)
from concourse.dve_uop import DveOpSpec

F32 = mybir.dt.float32
F16 = mybir.dt.float16
MMDT = mybir.dt.float16
AF = mybir.ActivationFunctionType
OP = mybir.AluOpType

B = 8
NB = 8
ND3 = 9
O_OUT = 100
IMGS = [[0, 1, 2, 3], [4, 5, 6, 7]]  # half -> image ids (all layers)


# ------------------------------------------------------------- custom DVE ops
def _register_dve_op(name, spec, subdim=False):
    if name in dve_ops._SUB_OPCODE_FOR_NAME:
        return next(op for op in dve_ops.OPS if op.name == name)
    row = max(dve_ops._SUB_OPCODE_FOR_NAME.values()) + 1
    assert row < 0x20
    ver = 'v3'
    tmp = DveOpSpec(name=name, opcode=row, uops=lower(spec, ver=ver),
                    rd1_en=_has_src1(spec))
    op = dve_ops.DveOp(name, spec, subdim, uops_sha={ver: tmp.sha(ver)})
    dve_ops.OPS.append(op)
    dve_ops.CUSTOM_DVE_SPECS[name] = spec
    dve_ops._SUB_OPCODE_FOR_NAME[name] = row
    return op


def _clamp_cube(in0, s0, s1):
    y = in0.astype(np.float32)
    m = np.maximum(np.minimum(y + s0, 0.0), s1)
    return m * m * m


def _d3p1_ref(in0, in1, s0, s1, imm2):
    return (imm2 * _clamp_cube(in0, s0, s1)).astype(np.float32)


def _d3p2_ref(in0, in1, s0, s1, imm2):
    return (in1.astype(np.float32) - _clamp_cube(in0, s0, s1)).astype(np.float32)


def _d3p3_ref(in0, in1, s0, s1, imm2):
    return (in1.astype(np.float32)
            - imm2 * _clamp_cube(in0, s0, s1)).astype(np.float32)


_c1 = maxx(minn(Src0 + C0, Zero), C1)
D3P1 = _register_dve_op('D3P1', Spec(body=C2 * (sq(_c1) * _c1),
                                     reference=_d3p1_ref))
_c2 = maxx(minn(Src0 + C0, Zero), C1)
D3P2 = _register_dve_op('D3P2', Spec(body=Src1 - sq(_c2) * _c2,
                                     reference=_d3p2_ref))
_c3 = maxx(minn(Src0 + C0, Zero), C1)
D3P3 = _register_dve_op('D3P3', Spec(body=Src1 - C2 * (sq(_c3) * _c3),
                                     reference=_d3p3_ref))


# ------------------------------------------------------------- weight folding
def silu_np(x):
    return x / (1.0 + np.exp(-x))


def d3_fold(blk):
    """blk: [NB(j), ...]. Returns [9(m), ...] with w'_m = w_m - w_{m-1}."""
    out = np.zeros((ND3,) + blk.shape[1:], blk.dtype)
    out[0:NB] = blk
    out[1:ND3] -= blk
    return out


def fold_weights(wb1, ws1, wb2, ws2, wb3, ws3, lb, lc):
    out = {}
    W1 = np.zeros((96, 3 * 64), np.float32)
    for kyi in range(3):
        for kxi in range(3):
            W1[kyi * 32 + 0:kyi * 32 + 3, kxi * 64:(kxi + 1) * 64] = wb1[:, :, kyi, kxi].T
            blk = np.transpose(ws1[:, :, kyi, kxi].reshape(64, 3, NB), (2, 1, 0)) / 6.0
            W1[kyi * 32 + 3:kyi * 32 + 30, kxi * 64:(kxi + 1) * 64] = \
                d3_fold(blk).reshape(27, 64)
    out['w1'] = W1.astype(MM_NP)

    W2 = np.zeros((640, 9 * 128), np.float32)
    for kyi in range(3):
        for kxi in range(3):
            t = kyi * 3 + kxi
            blk = np.transpose(ws2[:, :, kyi, kxi].reshape(128, 64, NB), (2, 1, 0)) / 6.0
            blk9 = d3_fold(blk).reshape(576, 128)
            W2[0:512, t * 128:(t + 1) * 128] = blk9[0:512]
            W2[512:576, t * 128:(t + 1) * 128] = wb2[:, :, kyi, kxi].T
            W2[576:640, t * 128:(t + 1) * 128] = blk9[512:576]
    out['w2'] = W2.astype(MM_NP)

    W3 = np.zeros((1280, 9 * 64), np.float32)
    for kyi in range(3):
        for kxi in range(3):
            t = kyi * 3 + kxi
            blk = np.transpose(ws3[:, :, kyi, kxi].reshape(64, 128, NB), (2, 1, 0)) / 6.0
            W3[0:1152, t * 64:(t + 1) * 64] = d3_fold(blk).reshape(1152, 64)
            W3[1152:1280, t * 64:(t + 1) * 64] = wb3[:, :, kyi, kxi].T
    out['w3'] = W3.astype(MM_NP)

    # linear: m-pair packed stationaries (K=128); pr4 = [m8 | base]
    lc9 = d3_fold(np.transpose(lc.reshape(O_OUT, 64, 64, NB), (3, 0, 1, 2)) / 6.0)
    lb_r = lb.reshape(O_OUT, 64, 64)
    WLP = np.zeros((128, 4, 16, 5, O_OUT), np.float32)
    for p in range(4):
        for yi in range(16):
            yx = p * 16 + yi
            for pr in range(4):
                WLP[0:64, p, yi, pr, :] = lc9[2 * pr, :, :, yx].T
                WLP[64:128, p, yi, pr, :] = lc9[2 * pr + 1, :, :, yx].T
            WLP[0:64, p, yi, 4, :] = lc9[8, :, :, yx].T
            WLP[64:128, p, yi, 4, :] = lb_r[:, :, yx].T
    out['wlp'] = WLP.reshape(128, 4 * 16 * 5 * O_OUT).astype(MM_NP)
    return out


# ------------------------------------------------------------- basis emission
def emit_d3_start(nc, tpool, d3pool, src_ap, P, E, bias_tiles, name=""):
    """Scalar RY pass + D3 tile allocation; slabs emitted via emit_d3_slabs."""
    RY = tpool.tile([P, E], F32, tag="ry", name=f"RY{name}")
    nc.scalar.activation(RY[:], src_ap, AF.Relu, bias=bias_tiles['b55'][0:P, :],
                         scale=-2.5)
    D3 = d3pool.tile([P, ND3 * E], F16, tag="d3", name=f"D3{name}")
    return RY, D3


def emit_d3_slabs(nc, tpool, RY, D3, P, E, m0, m1, name=""):
    """Emit d3 slabs m0..m1: three fused single-uop DVE ops per slab."""
    for m in range(m0, m1):
        A = tpool.tile([P, E], F32, tag="a", name=f"A{name}_{m}")
        nc.vector._custom_dve(D3P1, out=A[:], in0=RY[:], s0=float(m - 10),
                              s1=-2.0, imm2=3.0)
        A2 = tpool.tile([P, E], F32, tag="a2", name=f"A2{name}_{m}")
        nc.vector._custom_dve(D3P2, out=A2[:], in0=RY[:], in1=A[:],
                              s0=float(m - 11), s1=-3.0)
        nc.vector._custom_dve(D3P3, out=D3[:, m * E:(m + 1) * E], in0=RY[:],
                              in1=A2[:], s0=float(m - 9), s1=-1.0, imm2=3.0)


def emit_d3(nc, tpool, d3pool, src_ap, P, E, bias_tiles, name=""):
    RY, D3 = emit_d3_start(nc, tpool, d3pool, src_ap, P, E, bias_tiles, name)
    emit_d3_slabs(nc, tpool, RY, D3, P, E, 0, ND3, name)
    return D3


def maxpool_from_psum(nc, psum_ap, n_bh, W_half, out_ap):
    pv = psum_ap.rearrange("p (hp r2 wp c2) -> p hp wp r2 c2",
                           hp=n_bh, r2=2, wp=W_half, c2=2)
    nc.vector.tensor_reduce(out_ap.rearrange("p (hp wp) -> p hp wp", wp=W_half),
                            pv, mybir.AxisListType.XY, OP.max, opt_input=False)


# ----------------------------------------------------------------- the kernel
def build_nc(dbg=()):
    nc = bacc.Bacc("TRN2", target_bir_lowering=False, debug=False, num_devices=8)
    x_ext = nc.declare_dram_parameter("x", [B, 3, 64, 64], F32, isOutput=False)
    w1_ext = nc.declare_dram_parameter("w1", [96, 192], MMDT, isOutput=False)
    w2_ext = nc.declare_dram_parameter("w2", [640, 1152], MMDT, isOutput=False)
    w3_ext = nc.declare_dram_parameter("w3", [1280, 576], MMDT, isOutput=False)
    wlp_ext = nc.declare_dram_parameter("wlp", [128, 32000], MMDT, isOutput=False)
    out_ext = nc.declare_dram_parameter("out", [B, O_OUT], F32, isOutput=True)

    dbg_exts = {}

    def dbg_tap(name, shape, dt=F32):
        if name in dbg:
            dbg_exts[name] = nc.declare_dram_parameter(f"dbg_{name}", shape, dt, isOutput=True)
            return dbg_exts[name]
        return None

    with tile.TileContext(nc) as tc, ExitStack() as ctx:
        persist = ctx.enter_context(tc.tile_pool(name="persist", bufs=1))
        wpool = ctx.enter_context(tc.tile_pool(name="wpool", bufs=1))
        tpool = ctx.enter_context(tc.tile_pool(name="tpool", bufs=2))
        d3pool = ctx.enter_context(tc.tile_pool(name="d3pool", bufs=2))
        l23w = ctx.enter_context(tc.tile_pool(name="l23w", bufs=1))
        consp = ctx.enter_context(tc.tile_pool(name="cons", bufs=1))
        psg = ctx.enter_context(tc.tile_pool(name="psg", bufs=1, space="PSUM"))

        bias_tiles = {}
        bt_55 = wpool.tile([128, 1], F32, tag="bias_55", name="bias55")
        nc.gpsimd.memset(bt_55[:], 5.5)
        bias_tiles['b55'] = bt_55
        zt = wpool.tile([30, 64], MMDT)
        nc.gpsimd.memset(zt[:], 0.0)

        h1h = [persist.tile([64, 4096], F16, tag=f"h1_{i}", name=f"h1_{i}")
               for i in range(2)]
        h2h = [persist.tile([128, 1024], F32, tag=f"h2_{i}", name=f"h2_{i}")
               for i in range(2)]
        h3 = persist.tile([64, 512], F32)

        w2sb = [l23w.tile([128, 1152], MMDT, tag=f"w2_{i}", name=f"w2sb{i}")
                for i in range(5)]
        w3sb = [l23w.tile([128, 576], MMDT, tag=f"w3_{i}", name=f"w3sb{i}")
                for i in range(10)]
        h1s = l23w.tile([128, 4096], F16)

        T2w = {}  # wave -> list of 5 T2 tiles

        # ================= L1 (prologue) =================
        l1ctx = ExitStack()
        l1p = l1ctx.enter_context(tc.tile_pool(name="l1pool", bufs=1))
        X1 = l1p.tile([128, 768], F32)
        for c in range(3):
            nc.sync.dma_start(
                X1[:, c * 256:(c + 1) * 256],
                x_ext.ap()[:, c, :, :].rearrange("b (g hh) w -> b g (hh w)", g=16))
        w1sb = wpool.tile([96, 192], MMDT)
        nc.sync.dma_start(w1sb[:], w1_ext.ap())
        # preload L2/L3/linear-stationary weights during the prologue
        for i in range(5):
            nc.sync.dma_start(w2sb[i][:], w2_ext.ap()[i * 128:(i + 1) * 128, :])
        for i in range(10):
            nc.sync.dma_start(w3sb[i][:], w3_ext.ap()[i * 128:(i + 1) * 128, :])

        sl1 = l1p.tile([128, 768], MMDT)
        nc.scalar.activation(sl1[:], X1[:], AF.Silu)
        D3_1 = emit_d3(nc, tpool, d3pool, X1[:], 128, 768, bias_tiles,
                       name="l1")

        # dump channels to DRAM (ch-major), then read back partition-parallel
        # into the three ky-shifted 30-row blocks (K=90).
        l1ch = nc.dram_tensor("l1ch", [30, B * 4096], MMDT)
        for c in range(3):
            nc.sync.dma_start(
                l1ch.ap()[c, :].rearrange("(bg e) -> bg e", e=256),
                sl1[:, c * 256:(c + 1) * 256])
        for m in range(ND3):
            nc.sync.dma_start(
                l1ch.ap()[3 + m * 3:3 + m * 3 + 3, :]
                    .rearrange("c (bg e) -> bg c e", e=256),
                D3_1[:, m * 768:(m + 1) * 768]
                    .rearrange("p (c e) -> p c e", e=256))

        def l1_pair(pi):
            # pair pi covers images (2pi, 2pi+1); half bh = pi // 2
            bh, sp = pi // 2, pi % 2
            b0 = 2 * pi
            Bun = l1p.tile([96, 64 + 2 * 4096 + 64], MMDT, tag="bun1",
                           bufs=2, name=f"Bun1_{pi}")
            for kyi in range(3):
                base = 64 + (1 - kyi) * 64
                if pi == 0:
                    # split reads so the first half starts as early dumps land
                    nc.sync.dma_start(
                        Bun[kyi * 32:kyi * 32 + 15, base:base + 8192],
                        l1ch.ap()[0:15, b0 * 4096:(b0 + 2) * 4096])
                    nc.sync.dma_start(
                        Bun[kyi * 32 + 15:kyi * 32 + 30, base:base + 8192],
                        l1ch.ap()[15:30, b0 * 4096:(b0 + 2) * 4096])
                else:
                    nc.sync.dma_start(
                        Bun[kyi * 32:kyi * 32 + 30, base:base + 8192],
                        l1ch.ap()[:, b0 * 4096:(b0 + 2) * 4096])
                # pad rows 30:32: duplicate finite data (zero weights kill it)
                nc.sync.dma_start(
                    Bun[kyi * 32 + 30:kyi * 32 + 32, base:base + 8192],
                    l1ch.ap()[0:2, b0 * 4096:(b0 + 2) * 4096])
            # boundary rows: ky=0 block box-row 0; ky=2 block box-row 63
            for bi in range(2):
                nc.scalar.dma_start(
                    Bun[0:30, 64 + bi * 4096: 64 + bi * 4096 + 64], zt[:])
                nc.scalar.dma_start(
                    Bun[64:94, 64 + bi * 4096 + 63 * 64: 64 + bi * 4096 + 64 * 64],
                    zt[:])

            bun_v = Bun[:, 64:64 + 2 * 4096].rearrange(
                "p (b r w) -> p b r w", b=2, w=64)
            chunks = [(bi, hb) for bi in range(2) for hb in range(8)]
            for g in range(0, len(chunks), 8):
                grp = chunks[g:g + 8]
                pss = [psg.tile([64, 512], F32, tag="ps", bufs=8,
                                name=f"ps1_{pi}_{g}_{i}")
                       for i in range(len(grp))]
                for ti, kxi in enumerate([1, 0, 2]):
                    for ci, (bi, hb) in enumerate(grp):
                        ps = pss[ci]
                        if kxi == 0:
                            mv = bun_v[:, bi, hb * 8:hb * 8 + 8, 0:63]
                            ov = ps[:].rearrange("p (r w) -> p r w", w=64)[:, :, 1:64]
                        elif kxi == 1:
                            mv = bun_v[:, bi, hb * 8:hb * 8 + 8, :]
                            ov = ps[:]
                        else:
                            mv = bun_v[:, bi, hb * 8:hb * 8 + 8, 1:64]
                            ov = ps[:].rearrange("p (r w) -> p r w", w=64)[:, :, 0:63]
                        nc.tensor.matmul(ov, w1sb[:, kxi * 64:(kxi + 1) * 64], mv,
                                         start=(ti == 0), stop=(ti == 2))
                for ci, (bi, hb) in enumerate(grp):
                    slot = 2 * sp + bi
                    maxpool_from_psum(nc, pss[ci][:], 4, 32,
                                      h1h[bh][:, slot * 1024 + hb * 128:
                                              slot * 1024 + (hb + 1) * 128])

        # ================= L2 / L3 bodies =================
        W0S = {}

        def l2_prep_start(wv):
            bh, par = wv // 2, wv % 2
            col0 = bh * 2048 + par * 1024
            if par == 0:
                # issue on gpsimd (SWDGE) to keep the sync queue free for buns
                nc.gpsimd.dma_start(h1s[0:64, bh * 2048:(bh + 1) * 2048],
                                    h1h[bh][:, 0:2048])
                nc.gpsimd.dma_start(h1s[64:128, bh * 2048:(bh + 1) * 2048],
                                    h1h[bh][:, 2048:4096])
            T2 = [consp.tile([128, 2048], MMDT, tag=f"t2_{i}_{par}",
                             name=f"T2_{i}_{par}") for i in range(5)]
            T2w[wv] = T2
            RY, D3t = emit_d3_start(nc, tpool, d3pool, h1s[:, col0:col0 + 1024],
                                    128, 1024, bias_tiles, name=f"l2_{wv}")
            W0S[wv] = (RY, D3t)

        def l2_prep_slabs(wv, m0, m1):
            E = 1024
            RY, D3t = W0S[wv]
            T2 = T2w[wv]
            emit_d3_slabs(nc, tpool, RY, D3t, 128, E, m0, m1, name=f"l2_{wv}")
            for m in range(m0, m1):
                for ph in range(2):
                    if m < NB:
                        dst = T2[m // 2][(m % 2) * 64:(m % 2) * 64 + 64,
                                         ph * 1024:(ph + 1) * 1024]
                    else:
                        dst = T2[4][64:128, ph * 1024:(ph + 1) * 1024]
                    nc.scalar.dma_start(dst,
                                        D3t[ph * 64:(ph + 1) * 64,
                                            m * E:(m + 1) * E])

        def l2_prep_fin(wv):
            bh, par = wv // 2, wv % 2
            T2 = T2w[wv]
            del W0S[wv]
            # base silu -> T2[4] rows 0:64 (partition-aligned with h1h)
            for ii in range(2):
                slot = par + 2 * ii
                nc.scalar.activation(T2[4][0:64, ii * 1024:(ii + 1) * 1024],
                                     h1h[bh][:, slot * 1024:(slot + 1) * 1024],
                                     AF.Silu)

        def l2_prep(wv):
            l2_prep_start(wv)
            l2_prep_slabs(wv, 0, ND3)
            l2_prep_fin(wv)

        def l2_mm(wv):
            bh, par = wv // 2, wv % 2
            T2 = T2w.pop(wv)
            t2v = [T2[i][:].rearrange("p (b h w) -> p b h w", b=2, w=32)
                   for i in range(5)]
            taps = [(0, 1, 1)] + [(kt, kyi, kxi) for kt in range(5)
                                  for kyi in range(3) for kxi in range(3)
                                  if (kt, kyi, kxi) != (0, 1, 1)]
            n_taps = len(taps)
            chunks = [(ii, half) for ii in range(2) for half in range(2)]
            pss = [psg.tile([128, 512], F32, tag="ps", bufs=8,
                            name=f"ps2_{wv}_{i}") for i in range(4)]
            for tapi, (kt, kyi, kxi) in enumerate(taps):
                for ci, (ii, half) in enumerate(chunks):
                    h0 = half * 16
                    ps = pss[ci]
                    r_lo = max(0, 1 - kyi - h0)
                    r_hi = min(16, 33 - h0 - kyi)
                    w_lo = 1 if kxi == 0 else 0
                    w_hi = 31 if kxi == 2 else 32
                    in_row = h0 + r_lo + kyi - 1
                    in_col = w_lo + kxi - 1
                    mv = t2v[kt][:, ii, in_row:in_row + (r_hi - r_lo),
                                 in_col:in_col + (w_hi - w_lo)]
                    ov = ps[:].rearrange("p (r w) -> p r w", w=32)[
                        :, r_lo:r_hi, w_lo:w_hi]
                    nc.tensor.matmul(
                        ov,
                        w2sb[kt][:, (kyi * 3 + kxi) * 128:
                                 (kyi * 3 + kxi + 1) * 128],
                        mv, start=(tapi == 0),
                        stop=(tapi == n_taps - 1))
            for ci, (ii, half) in enumerate(chunks):
                slot = par + 2 * ii
                maxpool_from_psum(nc, pss[ci][:], 8, 16,
                                  h2h[bh][:, slot * 256 + half * 128:
                                          slot * 256 + (half + 1) * 128])

        D3l3 = {}

        def l3_prep(bh):
            E = 1024
            D3t = emit_d3(nc, tpool, d3pool, h2h[bh][:], 128, E,
                          bias_tiles, name="l3")
            Ts3 = consp.tile([128, 1024], MMDT, tag=f"ts3_{bh}", name=f"Ts3_{bh}")
            for p in range(2):
                nc.scalar.activation(Ts3[:, p * 512:(p + 1) * 512],
                                     h2h[bh][:, p * 512:(p + 1) * 512], AF.Silu)
            D3l3[bh] = (D3t, Ts3)

        def l3_mm(bh):
            imgs = IMGS[bh]
            E = 1024
            D3t, Ts3 = D3l3.pop(bh)
            t3v = [D3t[:, m * E:(m + 1) * E].rearrange(
                       "p (b h w) -> p b h w", b=4, w=16)
                   for m in range(ND3)]
            ts3v = Ts3[:].rearrange("p (b h w) -> p b h w", b=4, w=16)
            taps = [(0, 1, 1)] + [(kt, kyi, kxi) for kt in range(10)
                                  for kyi in range(3) for kxi in range(3)
                                  if (kt, kyi, kxi) != (0, 1, 1)]
            n_taps = len(taps)
            pss = [psg.tile([64, 512], F32, tag="ps", bufs=8,
                            name=f"ps3_{bh}_{i}") for i in range(2)]
            for tapi, (kt, kyi, kxi) in enumerate(taps):
                for ckc in range(2):
                    b0c = ckc * 2
                    ps = pss[ckc]
                    r_lo = max(0, 1 - kyi)
                    r_hi = min(16, 17 - kyi)
                    w_lo = 1 if kxi == 0 else 0
                    w_hi = 15 if kxi == 2 else 16
                    src_ = t3v[kt] if kt < ND3 else ts3v
                    mv = src_[:, b0c:b0c + 2, r_lo + kyi - 1:r_hi + kyi - 1,
                              w_lo + kxi - 1:w_lo + kxi - 1 + (w_hi - w_lo)]
                    ov = ps[:].rearrange("p (b r w) -> p b r w", b=2, w=16)[
                        :, :, r_lo:r_hi, w_lo:w_hi]
                    nc.tensor.matmul(
                        ov,
                        w3sb[kt][:, (kyi * 3 + kxi) * 64:
                                 (kyi * 3 + kxi + 1) * 64],
                        mv, start=(tapi == 0),
                        stop=(tapi == n_taps - 1))
            for ckc in range(2):
                b0i = imgs[2 * ckc]
                maxpool_from_psum(nc, pss[ckc][:], 16, 8,
                                  h3[:, b0i * 64:(b0i + 2) * 64])

        # ================= Linear bodies =================
        linctx = ExitStack()

        def lin_open():
            # l1pool space is free by now; reuse it for the linear tiles
            lin = {}
            lin['p'] = linctx.enter_context(tc.tile_pool(name="linp", bufs=1))
            lin['sl3'] = lin['p'].tile([64, 512], MMDT, name="sl3")
            lin['D3P'] = lin['p'].tile([128, 5 * 512], MMDT, name="D3P")
            return lin

        def lin_prep(lin, half):
            # emit basis for images [4*half, 4*half+4) (contiguous h3 cols)
            sl3 = lin['sl3']
            nc.scalar.activation(sl3[:, half * 256:(half + 1) * 256],
                                 h3[:, half * 256:(half + 1) * 256], AF.Silu)
            D3L = emit_d3(nc, tpool, d3pool, h3[:, half * 256:(half + 1) * 256],
                          64, 256, bias_tiles, name=f"lin{half}")
            d3l_v = D3L[:].rearrange("p (m b yx) -> p m b yx", m=ND3, b=4)
            d3p_v = lin['D3P'][:].rearrange("p (pr b yx) -> p pr b yx", pr=5, b=B)
            bsl = slice(half * 4, half * 4 + 4)
            nc.sync.dma_start(d3p_v[0:64, 0:4, bsl, :], d3l_v[:, 0:8:2, :, :])
            nc.sync.dma_start(d3p_v[64:128, 0:4, bsl, :], d3l_v[:, 1:8:2, :, :])
            nc.sync.dma_start(d3p_v[0:64, 4, bsl, :], d3l_v[:, 8, :, :])
            nc.sync.dma_start(
                d3p_v[64:128, 4, bsl, :],
                sl3[:, half * 256:(half + 1) * 256]
                    .rearrange("p (b yx) -> p b yx", b=4))

        def lin_mm(lin):
            d3p_v = lin['D3P'][:].rearrange("p (pr b yx) -> p pr b yx", pr=5, b=B)
            with tc.tile_pool(name="wlpool", bufs=2) as wlp:
                psl = psg.tile([B, O_OUT], F32, tag="ps", bufs=8, name="psl")
                first = True
                for piece in range(4):
                    wlt = wlp.tile([128, 8000], MMDT, tag="wl_piece", name="wlt")
                    nc.sync.dma_start(wlt[:], wlp_ext.ap()[:, piece * 8000:(piece + 1) * 8000])
                    for yi in range(16):
                        for pr in range(5):
                            nc.tensor.matmul(
                                psl[:], d3p_v[:, pr, :, piece * 16 + yi],
                                wlt[:, (yi * 5 + pr) * O_OUT:(yi * 5 + pr + 1) * O_OUT],
                                start=first,
                                stop=(piece == 3 and yi == 15 and pr == 4))
                            first = False
                osb = lin['p'].tile([B, O_OUT], F32, name="osb")
                nc.vector.tensor_copy(osb[:], psl[:])
                nc.sync.dma_start(out_ext.ap(), osb[:])

        # ================= schedule =================
        l1_pair(0)
        l1_pair(1)
        l2_prep_start(0)
        l2_prep_slabs(0, 0, 3)
        l1_pair(2)
        l2_prep_slabs(0, 3, 6)
        l1_pair(3)
        l2_prep_slabs(0, 6, 9)
        l2_prep_fin(0)
        l2_prep(1)
        l1ctx.close()
        lin = lin_open()
        l2_mm(0)
        l2_prep(2)
        l2_mm(1)
        l2_prep(3)
        l2_mm(2)
        l3_prep(0)
        l2_mm(3)
        l3_mm(0)
        l3_prep(1)
        lin_prep(lin, 0)
        l3_mm(1)
        lin_prep(lin, 1)
        if (t := dbg_tap('h3', [64, 512])) is not None:
            nc.sync.dma_start(t.ap(), h3[:])
        lin_mm(lin)
        linctx.close()

    nc.compile()
    return nc

# ===================================================================== runner
from concourse.bass_utils import run_bass_kernel_spmd

_NC_CACHE = {}


def _get_nc():
    if 'nc' not in _NC_CACHE:
        _NC_CACHE['nc'] = build_nc(dbg=())
    return _NC_CACHE['nc']


def kernel(x, wb1, ws1, wb2, ws2, wb3, ws3, lb, lc):
    """Full-input entry point: x [64,3,64,64] f32 -> out [64,100] f32.
    Shards the batch over 8 NeuronCores (8 samples each), replicating weights."""
    x = np.ascontiguousarray(np.asarray(x, dtype=np.float32))
    w = fold_weights(np.asarray(wb1, np.float32), np.asarray(ws1, np.float32),
                     np.asarray(wb2, np.float32), np.asarray(ws2, np.float32),
                     np.asarray(wb3, np.float32), np.asarray(ws3, np.float32),
                     np.asarray(lb, np.float32), np.asarray(lc, np.float32))
    nc = _get_nc()
    in_maps = [{'x': x[i * B:(i + 1) * B], **w} for i in range(8)]
    res = run_bass_kernel_spmd(nc, in_maps, core_ids=list(range(8)))
    return np.concatenate([res.results[i]['out'] for i in range(8)], axis=0)


# revision 27
# speedup vs baseline: 1.0031x; 1.0031x over previous
"""CKANKANNet Trainium2 kernel (per-core SPMD program, B=8 samples/core).

Basis algorithm (d3-direct, weight-folded): with v = 2.5x+5.5 mapped to
integer knots and V = min(v, 11) via RY = relu(5.5-2.5x), the 3rd finite
difference of relu cubes saturates: d3_m(t) = t^3 - 3 relu(t-1)^3
+ 3 relu(t-2)^3 for t = clamp(V-m, 0, 3), constant 6 beyond. Each d3 slab
is THREE fused single-uop DVE ops straight from RY (clamps commuted into
the op bodies: u=max(min(RY+m-11,0),-3), m1=max(min(RY+m-10,0),-2),
m2=max(min(RY+m-9,0),-1); P1: 3*m1^3; P2: A - u^3; P3: A2 - 3*m2^3).
Since conv/linear are linear in the basis, the d4_j = d3_j - d3_{j+1}
difference is folded into the weights (w'_m = w_m - w_{m-1}, 9 channels):
no subtract passes, and L3/linear consume D3 tiles directly. The /6 is
folded into the weights too.

Convs: fp16 matmuls, channels on K partitions, 3x3 taps as accumulating
matmuls with edge-trimmed N ranges. L1 bakes ky taps into K via 3 shifted
channel-block copies (K=96, 30 live rows each). L2 K = 5 full 128-blocks
(4 m-pairs + [base|m8]). L3 K = 10 blocks (9 m + base). Image halves are
{0,1,4,5}/{2,3,6,7} at every layer so each layer's half-0 compute overlaps
the previous layer's half-1. Linear uses m-pair packed K=128 stationaries
(pr4 = [m8|base]).
"""
import sys
sys.path.insert(0, '/opt/trn_rl_repo')
from contextlib import ExitStack

import numpy as np
MM_NP = np.float16

import concourse.bass as bass
import concourse.tile as tile
from concourse import bacc, mybir
from concourse import dve_ops
from concourse.dve_spec import (Spec, Src0, Src1, sq, lower, minn, maxx,
                                _has_src1, C0, C1, C2, Zero, One)
from concourse.dve_uop import DveOpSpec

F32 = mybir.dt.float32
F16 = mybir.dt.float16
MMDT = mybir.dt.float16
AF = mybir.ActivationFunctionType
OP = mybir.AluOpType

B = 8
NB = 8
ND3 = 9
O_OUT = 100
IMGS = [[0, 1, 2, 3], [4, 5, 6, 7]]  # half -> image ids (all layers)


# ------------------------------------------------------------- custom DVE ops
def _register_dve_op(name, spec, subdim=False):
    if name in dve_ops._SUB_OPCODE_FOR_NAME:
        return next(op for op in dve_ops.OPS if op.name == name)
    row = max(dve_ops._SUB_OPCODE_FOR_NAME.values()) + 1
    assert row < 0x20
    ver = 'v3'
    tmp = DveOpSpec(name=name, opcode=row, uops=lower(spec, ver=ver),
                    rd1_en=_has_src1(spec))
    op = dve_ops.DveOp(name, spec, subdim, uops_sha={ver: tmp.sha(ver)})
    dve_ops.OPS.append(op)
    dve_ops.CUSTOM_DVE_SPECS[name] = spec
    dve_ops._SUB_OPCODE_FOR_NAME[name] = row
    return op


def _clamp_cube(in0, s0, s1):
    y = in0.astype(np.float32)
    m = np.maximum(np.minimum(y + s0, 0.0), s1)
    return m * m * m


def _d3p1_ref(in0, in1, s0, s1, imm2):
    return (imm2 * _clamp_cube(in0, s0, s1)).astype(np.float32)


def _d3p2_ref(in0, in1, s0, s1, imm2):
    return (in1.astype(np.float32) - _clamp_cube(in0, s0, s1)).astype(np.float32)


def _d3p3_ref(in0, in1, s0, s1, imm2):
    return (in1.astype(np.float32)
            - imm2 * _clamp_cube(in0, s0, s1)).astype(np.float32)


_c1 = maxx(minn(Src0 + C0, Zero), C1)
D3P1 = _register_dve_op('D3P1', Spec(body=C2 * (sq(_c1) * _c1),
                                     reference=_d3p1_ref))
_c2 = maxx(minn(Src0 + C0, Zero), C1)
D3P2 = _register_dve_op('D3P2', Spec(body=Src1 - sq(_c2) * _c2,
                                     reference=_d3p2_ref))
_c3 = maxx(minn(Src0 + C0, Zero), C1)
D3P3 = _register_dve_op('D3P3', Spec(body=Src1 - C2 * (sq(_c3) * _c3),
                                     reference=_d3p3_ref))


# ------------------------------------------------------------- weight folding
def silu_np(x):
    return x / (1.0 + np.exp(-x))


def d3_fold(blk):
    """blk: [NB(j), ...]. Returns [9(m), ...] with w'_m = w_m - w_{m-1}."""
    out = np.zeros((ND3,) + blk.shape[1:], blk.dtype)
    out[0:NB] = blk
    out[1:ND3] -= blk
    return out


def fold_weights(wb1, ws1, wb2, ws2, wb3, ws3, lb, lc):
    out = {}
    W1 = np.zeros((96, 3 * 64), np.float32)
    for kyi in range(3):
        for kxi in range(3):
            W1[kyi * 32 + 0:kyi * 32 + 3, kxi * 64:(kxi + 1) * 64] = wb1[:, :, kyi, kxi].T
            blk = np.transpose(ws1[:, :, kyi, kxi].reshape(64, 3, NB), (2, 1, 0)) / 6.0
            W1[kyi * 32 + 3:kyi * 32 + 30, kxi * 64:(kxi + 1) * 64] = \
                d3_fold(blk).reshape(27, 64)
    out['w1'] = W1.astype(MM_NP)

    W2 = np.zeros((640, 9 * 128), np.float32)
    for kyi in range(3):
        for kxi in range(3):
            t = kyi * 3 + kxi
            blk = np.transpose(ws2[:, :, kyi, kxi].reshape(128, 64, NB), (2, 1, 0)) / 6.0
            blk9 = d3_fold(blk).reshape(576, 128)
            W2[0:512, t * 128:(t + 1) * 128] = blk9[0:512]
            W2[512:576, t * 128:(t + 1) * 128] = wb2[:, :, kyi, kxi].T
            W2[576:640, t * 128:(t + 1) * 128] = blk9[512:576]
    out['w2'] = W2.astype(MM_NP)

    W3 = np.zeros((1280, 9 * 64), np.float32)
    for kyi in range(3):
        for kxi in range(3):
            t = kyi * 3 + kxi
            blk = np.transpose(ws3[:, :, kyi, kxi].reshape(64, 128, NB), (2, 1, 0)) / 6.0
            W3[0:1152, t * 64:(t + 1) * 64] = d3_fold(blk).reshape(1152, 64)
            W3[1152:1280, t * 64:(t + 1) * 64] = wb3[:, :, kyi, kxi].T
    out['w3'] = W3.astype(MM_NP)

    # linear: m-pair packed stationaries (K=128); pr4 = [m8 | base]
    lc9 = d3_fold(np.transpose(lc.reshape(O_OUT, 64, 64, NB), (3, 0, 1, 2)) / 6.0)
    lb_r = lb.reshape(O_OUT, 64, 64)
    WLP = np.zeros((128, 4, 16, 5, O_OUT), np.float32)
    for p in range(4):
        for yi in range(16):
            yx = p * 16 + yi
            for pr in range(4):
                WLP[0:64, p, yi, pr, :] = lc9[2 * pr, :, :, yx].T
                WLP[64:128, p, yi, pr, :] = lc9[2 * pr + 1, :, :, yx].T
            WLP[0:64, p, yi, 4, :] = lc9[8, :, :, yx].T
            WLP[64:128, p, yi, 4, :] = lb_r[:, :, yx].T
    out['wlp'] = WLP.reshape(128, 4 * 16 * 5 * O_OUT).astype(MM_NP)
    return out


# ------------------------------------------------------------- basis emission
def emit_d3_start(nc, tpool, d3pool, src_ap, P, E, bias_tiles, name=""):
    """Scalar RY pass + D3 tile allocation; slabs emitted via emit_d3_slabs."""
    RY = tpool.tile([P, E], F32, tag="ry", name=f"RY{name}")
    nc.scalar.activation(RY[:], src_ap, AF.Relu, bias=bias_tiles['b55'][0:P, :],
                         scale=-2.5)
    D3 = d3pool.tile([P, ND3 * E], F16, tag="d3", name=f"D3{name}")
    return RY, D3


def emit_d3_slabs(nc, tpool, RY, D3, P, E, m0, m1, name=""):
    """Emit d3 slabs m0..m1: three fused single-uop DVE ops per slab."""
    for m in range(m0, m1):
        A = tpool.tile([P, E], F32, tag="a", name=f"A{name}_{m}")
        nc.vector._custom_dve(D3P1, out=A[:], in0=RY[:], s0=float(m - 10),
                              s1=-2.0, imm2=3.0)
        A2 = tpool.tile([P, E], F32, tag="a2", name=f"A2{name}_{m}")
        nc.vector._custom_dve(D3P2, out=A2[:], in0=RY[:], in1=A[:],
                              s0=float(m - 11), s1=-3.0)
        nc.vector._custom_dve(D3P3, out=D3[:, m * E:(m + 1) * E], in0=RY[:],
                              in1=A2[:], s0=float(m - 9), s1=-1.0, imm2=3.0)


def emit_d3(nc, tpool, d3pool, src_ap, P, E, bias_tiles, name=""):
    RY, D3 = emit_d3_start(nc, tpool, d3pool, src_ap, P, E, bias_tiles, name)
    emit_d3_slabs(nc, tpool, RY, D3, P, E, 0, ND3, name)
    return D3


def maxpool_from_psum(nc, psum_ap, n_bh, W_half, out_ap):
    pv = psum_ap.rearrange("p (hp r2 wp c2) -> p hp wp r2 c2",
                           hp=n_bh, r2=2, wp=W_half, c2=2)
    nc.vector.tensor_reduce(out_ap.rearrange("p (hp wp) -> p hp wp", wp=W_half),
                            pv, mybir.AxisListType.XY, OP.max, opt_input=False)


# ----------------------------------------------------------------- the kernel
def build_nc(dbg=()):
    nc = bacc.Bacc("TRN2", target_bir_lowering=False, debug=False, num_devices=8)
    x_ext = nc.declare_dram_parameter("x", [B, 3, 64, 64], F32, isOutput=False)
    w1_ext = nc.declare_dram_parameter("w1", [96, 192], MMDT, isOutput=False)
    w2_ext = nc.declare_dram_parameter("w2", [640, 1152], MMDT, isOutput=False)
    w3_ext = nc.declare_dram_parameter("w3", [1280, 576], MMDT, isOutput=False)
    wlp_ext = nc.declare_dram_parameter("wlp", [128, 32000], MMDT, isOutput=False)
    out_ext = nc.declare_dram_parameter("out", [B, O_OUT], F32, isOutput=True)

    dbg_exts = {}

    def dbg_tap(name, shape, dt=F32):
        if name in dbg:
            dbg_exts[name] = nc.declare_dram_parameter(f"dbg_{name}", shape, dt, isOutput=True)
            return dbg_exts[name]
        return None

    with tile.TileContext(nc) as tc, ExitStack() as ctx:
        persist = ctx.enter_context(tc.tile_pool(name="persist", bufs=1))
        wpool = ctx.enter_context(tc.tile_pool(name="wpool", bufs=1))
        tpool = ctx.enter_context(tc.tile_pool(name="tpool", bufs=2))
        d3pool = ctx.enter_context(tc.tile_pool(name="d3pool", bufs=2))
        l23w = ctx.enter_context(tc.tile_pool(name="l23w", bufs=1))
        consp = ctx.enter_context(tc.tile_pool(name="cons", bufs=1))
        psg = ctx.enter_context(tc.tile_pool(name="psg", bufs=1, space="PSUM"))

        bias_tiles = {}
        bt_55 = wpool.tile([128, 1], F32, tag="bias_55", name="bias55")
        nc.gpsimd.memset(bt_55[:], 5.5)
        bias_tiles['b55'] = bt_55
        zt = wpool.tile([30, 64], MMDT)
        nc.gpsimd.memset(zt[:], 0.0)

        h2h = [persist.tile([128, 1024], F32, tag=f"h2_{i}", name=f"h2_{i}")
               for i in range(2)]
        h3 = persist.tile([64, 512], F32)

        w2sb = [l23w.tile([128, 1152], MMDT, tag=f"w2_{i}", name=f"w2sb{i}")
                for i in range(5)]
        w3sb = [l23w.tile([128, 576], MMDT, tag=f"w3_{i}", name=f"w3sb{i}")
                for i in range(10)]
        h1s = l23w.tile([128, 4096], F16)

        T2w = {}  # wave -> list of 5 T2 tiles

        # ================= L1 (prologue) =================
        l1ctx = ExitStack()
        l1p = l1ctx.enter_context(tc.tile_pool(name="l1pool", bufs=1))
        X1 = l1p.tile([128, 768], F32)
        for c in range(3):
            nc.sync.dma_start(
                X1[:, c * 256:(c + 1) * 256],
                x_ext.ap()[:, c, :, :].rearrange("b (g hh) w -> b g (hh w)", g=16))
        w1sb = wpool.tile([96, 192], MMDT)
        nc.sync.dma_start(w1sb[:], w1_ext.ap())
        # preload L2/L3/linear-stationary weights during the prologue
        for i in range(5):
            nc.sync.dma_start(w2sb[i][:], w2_ext.ap()[i * 128:(i + 1) * 128, :])
        for i in range(10):
            nc.sync.dma_start(w3sb[i][:], w3_ext.ap()[i * 128:(i + 1) * 128, :])

        sl1 = l1p.tile([128, 768], MMDT)
        nc.scalar.activation(sl1[:], X1[:], AF.Silu)
        D3_1 = emit_d3(nc, tpool, d3pool, X1[:], 128, 768, bias_tiles,
                       name="l1")

        # dump channels to DRAM (ch-major), then read back partition-parallel
        # into the three ky-shifted 30-row blocks. Pair-0 reads are split and
        # interleaved with the dumps so its bun assembles as slabs land.
        l1ch = nc.dram_tensor("l1ch", [30, B * 4096], MMDT)
        for c in range(3):
            nc.sync.dma_start(
                l1ch.ap()[c, :].rearrange("(bg e) -> bg e", e=256),
                sl1[:, c * 256:(c + 1) * 256])
        for m in range(4):
            nc.sync.dma_start(
                l1ch.ap()[3 + m * 3:3 + m * 3 + 3, :]
                    .rearrange("c (bg e) -> bg c e", e=256),
                D3_1[:, m * 768:(m + 1) * 768]
                    .rearrange("p (c e) -> p c e", e=256))
        Bun0 = l1p.tile([96, 64 + 2 * 4096 + 64], MMDT, tag="bun1",
                        bufs=2, name="Bun1_0")
        for kyi in range(3):
            base = 64 + (1 - kyi) * 64
            nc.sync.dma_start(Bun0[kyi * 32:kyi * 32 + 15, base:base + 8192],
                              l1ch.ap()[0:15, 0:8192])
            nc.sync.dma_start(
                Bun0[kyi * 32 + 30:kyi * 32 + 32, base:base + 8192],
                l1ch.ap()[0:2, 0:8192])
        for m in range(4, ND3):
            nc.sync.dma_start(
                l1ch.ap()[3 + m * 3:3 + m * 3 + 3, :]
                    .rearrange("c (bg e) -> bg c e", e=256),
                D3_1[:, m * 768:(m + 1) * 768]
                    .rearrange("p (c e) -> p c e", e=256))
        for kyi in range(3):
            base = 64 + (1 - kyi) * 64
            nc.sync.dma_start(Bun0[kyi * 32 + 15:kyi * 32 + 30, base:base + 8192],
                              l1ch.ap()[15:30, 0:8192])

        def l1_pair(pi):
            # pair pi covers images (2pi, 2pi+1) == L2 wave pi's image pair
            b0 = 2 * pi
            if pi == 0:
                Bun = Bun0
            else:
                Bun = l1p.tile([96, 64 + 2 * 4096 + 64], MMDT, tag="bun1",
                               bufs=2, name=f"Bun1_{pi}")
                for kyi in range(3):
                    base = 64 + (1 - kyi) * 64
                    nc.sync.dma_start(
                        Bun[kyi * 32:kyi * 32 + 30, base:base + 8192],
                        l1ch.ap()[:, b0 * 4096:(b0 + 2) * 4096])
                    # pad rows 30:32: duplicate finite data (zero weights)
                    nc.sync.dma_start(
                        Bun[kyi * 32 + 30:kyi * 32 + 32, base:base + 8192],
                        l1ch.ap()[0:2, b0 * 4096:(b0 + 2) * 4096])
            # boundary rows: ky=0 block box-row 0; ky=2 block box-row 63
            for bi in range(2):
                nc.sync.dma_start(
                    Bun[0:30, 64 + bi * 4096: 64 + bi * 4096 + 64], zt[:])
                nc.sync.dma_start(
                    Bun[64:94, 64 + bi * 4096 + 63 * 64: 64 + bi * 4096 + 64 * 64],
                    zt[:])

            bun_v = Bun[:, 64:64 + 2 * 4096].rearrange(
                "p (b r w) -> p b r w", b=2, w=64)
            for g in range(2):
                pss = [psg.tile([128, 512], F32, tag="ps", bufs=8,
                                name=f"ps1_{pi}_{g}_{i}") for i in range(4)]
                for ti, kxi in enumerate([1, 0, 2]):
                    for bi in range(2):
                        for hbi in range(4):
                            hb = g * 4 + hbi
                            ps = pss[hbi]
                            if kxi == 0:
                                mv = bun_v[:, bi, hb * 8:hb * 8 + 8, 0:63]
                                ov = ps[bi * 64:bi * 64 + 64].rearrange(
                                    "p (r w) -> p r w", w=64)[:, :, 1:64]
                            elif kxi == 1:
                                mv = bun_v[:, bi, hb * 8:hb * 8 + 8, :]
                                ov = ps[bi * 64:bi * 64 + 64]
                            else:
                                mv = bun_v[:, bi, hb * 8:hb * 8 + 8, 1:64]
                                ov = ps[bi * 64:bi * 64 + 64].rearrange(
                                    "p (r w) -> p r w", w=64)[:, :, 0:63]
                            nc.tensor.matmul(ov, w1sb[:, kxi * 64:(kxi + 1) * 64],
                                             mv, start=(ti == 0), stop=(ti == 2))
                for hbi in range(4):
                    hb = g * 4 + hbi
                    maxpool_from_psum(nc, pss[hbi][:], 4, 32,
                                      h1s[:, pi * 1024 + hb * 128:
                                          pi * 1024 + (hb + 1) * 128])

        # ================= L2 / L3 bodies =================
        W0S = {}

        def l2_prep_start(wv):
            T2 = [consp.tile([128, 2048], MMDT, tag=f"t2_{i}_{wv % 2}",
                             name=f"T2_{i}_{wv % 2}") for i in range(5)]
            T2w[wv] = T2
            RY, D3t = emit_d3_start(nc, tpool, d3pool,
                                    h1s[:, wv * 1024:(wv + 1) * 1024],
                                    128, 1024, bias_tiles, name=f"l2_{wv}")
            W0S[wv] = (RY, D3t)

        def l2_prep_slabs(wv, m0, m1):
            E = 1024
            RY, D3t = W0S[wv]
            T2 = T2w[wv]
            emit_d3_slabs(nc, tpool, RY, D3t, 128, E, m0, m1, name=f"l2_{wv}")
            for m in range(m0, m1):
                for ph in range(2):
                    if m < NB:
                        dst = T2[m // 2][(m % 2) * 64:(m % 2) * 64 + 64,
                                         ph * 1024:(ph + 1) * 1024]
                    else:
                        dst = T2[4][64:128, ph * 1024:(ph + 1) * 1024]
                    nc.scalar.dma_start(dst,
                                        D3t[ph * 64:(ph + 1) * 64,
                                            m * E:(m + 1) * E])

        def l2_prep_fin(wv):
            T2 = T2w[wv]
            del W0S[wv]
            # base silu -> T2[4] rows 0:64; pair img 0 direct, img 1 via temp
            nc.scalar.activation(T2[4][0:64, 0:1024],
                                 h1s[0:64, wv * 1024:(wv + 1) * 1024], AF.Silu)
            TS = tpool.tile([128, 1024], MMDT, tag="ts2", name=f"TS{wv}")
            nc.scalar.activation(TS[64:128, :],
                                 h1s[64:128, wv * 1024:(wv + 1) * 1024], AF.Silu)
            nc.scalar.dma_start(T2[4][0:64, 1024:2048], TS[64:128, :])

        def l2_prep(wv):
            l2_prep_start(wv)
            l2_prep_slabs(wv, 0, ND3)
            l2_prep_fin(wv)

        def l2_mm(wv):
            bh, par = wv // 2, wv % 2
            T2 = T2w.pop(wv)
            t2v = [T2[i][:].rearrange("p (b h w) -> p b h w", b=2, w=32)
                   for i in range(5)]
            taps = [(0, 1, 1)] + [(kt, kyi, kxi) for kt in range(5)
                                  for kyi in range(3) for kxi in range(3)
                                  if (kt, kyi, kxi) != (0, 1, 1)]
            n_taps = len(taps)
            chunks = [(ii, half) for ii in range(2) for half in range(2)]
            pss = [psg.tile([128, 512], F32, tag="ps", bufs=8,
                            name=f"ps2_{wv}_{i}") for i in range(4)]
            for tapi, (kt, kyi, kxi) in enumerate(taps):
                for ci, (ii, half) in enumerate(chunks):
                    h0 = half * 16
                    ps = pss[ci]
                    r_lo = max(0, 1 - kyi - h0)
                    r_hi = min(16, 33 - h0 - kyi)
                    w_lo = 1 if kxi == 0 else 0
                    w_hi = 31 if kxi == 2 else 32
                    in_row = h0 + r_lo + kyi - 1
                    in_col = w_lo + kxi - 1
                    mv = t2v[kt][:, ii, in_row:in_row + (r_hi - r_lo),
                                 in_col:in_col + (w_hi - w_lo)]
                    ov = ps[:].rearrange("p (r w) -> p r w", w=32)[
                        :, r_lo:r_hi, w_lo:w_hi]
                    nc.tensor.matmul(
                        ov,
                        w2sb[kt][:, (kyi * 3 + kxi) * 128:
                                 (kyi * 3 + kxi + 1) * 128],
                        mv, start=(tapi == 0),
                        stop=(tapi == n_taps - 1))
            for ci, (ii, half) in enumerate(chunks):
                slot = 2 * par + ii
                maxpool_from_psum(nc, pss[ci][:], 8, 16,
                                  h2h[bh][:, slot * 256 + half * 128:
                                          slot * 256 + (half + 1) * 128])

        D3l3 = {}

        def l3_prep(bh):
            E = 1024
            D3t = emit_d3(nc, tpool, d3pool, h2h[bh][:], 128, E,
                          bias_tiles, name="l3")
            Ts3 = consp.tile([128, 1024], MMDT, tag=f"ts3_{bh}", name=f"Ts3_{bh}")
            for p in range(2):
                nc.scalar.activation(Ts3[:, p * 512:(p + 1) * 512],
                                     h2h[bh][:, p * 512:(p + 1) * 512], AF.Silu)
            D3l3[bh] = (D3t, Ts3)

        def l3_mm(bh):
            imgs = IMGS[bh]
            E = 1024
            D3t, Ts3 = D3l3.pop(bh)
            t3v = [D3t[:, m * E:(m + 1) * E].rearrange(
                       "p (b h w) -> p b h w", b=4, w=16)
                   for m in range(ND3)]
            ts3v = Ts3[:].rearrange("p (b h w) -> p b h w", b=4, w=16)
            taps = [(0, 1, 1)] + [(kt, kyi, kxi) for kt in range(10)
                                  for kyi in range(3) for kxi in range(3)
                                  if (kt, kyi, kxi) != (0, 1, 1)]
            n_taps = len(taps)
            pss = [psg.tile([64, 512], F32, tag="ps", bufs=8,
                            name=f"ps3_{bh}_{i}") for i in range(2)]
            for tapi, (kt, kyi, kxi) in enumerate(taps):
                for ckc in range(2):
                    b0c = ckc * 2
                    ps = pss[ckc]
                    r_lo = max(0, 1 - kyi)
                    r_hi = min(16, 17 - kyi)
                    w_lo = 1 if kxi == 0 else 0
                    w_hi = 15 if kxi == 2 else 16
                    src_ = t3v[kt] if kt < ND3 else ts3v
                    mv = src_[:, b0c:b0c + 2, r_lo + kyi - 1:r_hi + kyi - 1,
                              w_lo + kxi - 1:w_lo + kxi - 1 + (w_hi - w_lo)]
                    ov = ps[:].rearrange("p (b r w) -> p b r w", b=2, w=16)[
                        :, :, r_lo:r_hi, w_lo:w_hi]
                    nc.tensor.matmul(
                        ov,
                        w3sb[kt][:, (kyi * 3 + kxi) * 64:
                                 (kyi * 3 + kxi + 1) * 64],
                        mv, start=(tapi == 0),
                        stop=(tapi == n_taps - 1))
            for ckc in range(2):
                b0i = imgs[2 * ckc]
                maxpool_from_psum(nc, pss[ckc][:], 16, 8,
                                  h3[:, b0i * 64:(b0i + 2) * 64])

        # ================= Linear bodies =================
        linctx = ExitStack()

        def lin_open():
            # l1pool space is free by now; reuse it for the linear tiles
            lin = {}
            lin['p'] = linctx.enter_context(tc.tile_pool(name="linp", bufs=1))
            lin['sl3'] = lin['p'].tile([64, 512], MMDT, name="sl3")
            lin['D3P'] = lin['p'].tile([128, 5 * 512], MMDT, name="D3P")
            return lin

        def lin_prep(lin, half):
            # emit basis for images [4*half, 4*half+4) (contiguous h3 cols)
            sl3 = lin['sl3']
            nc.scalar.activation(sl3[:, half * 256:(half + 1) * 256],
                                 h3[:, half * 256:(half + 1) * 256], AF.Silu)
            D3L = emit_d3(nc, tpool, d3pool, h3[:, half * 256:(half + 1) * 256],
                          64, 256, bias_tiles, name=f"lin{half}")
            d3l_v = D3L[:].rearrange("p (m b yx) -> p m b yx", m=ND3, b=4)
            d3p_v = lin['D3P'][:].rearrange("p (pr b yx) -> p pr b yx", pr=5, b=B)
            bsl = slice(half * 4, half * 4 + 4)
            nc.sync.dma_start(d3p_v[0:64, 0:4, bsl, :], d3l_v[:, 0:8:2, :, :])
            nc.sync.dma_start(d3p_v[64:128, 0:4, bsl, :], d3l_v[:, 1:8:2, :, :])
            nc.sync.dma_start(d3p_v[0:64, 4, bsl, :], d3l_v[:, 8, :, :])
            nc.sync.dma_start(
                d3p_v[64:128, 4, bsl, :],
                sl3[:, half * 256:(half + 1) * 256]
                    .rearrange("p (b yx) -> p b yx", b=4))

        def lin_mm(lin):
            d3p_v = lin['D3P'][:].rearrange("p (pr b yx) -> p pr b yx", pr=5, b=B)
            with tc.tile_pool(name="wlpool", bufs=2) as wlp:
                psl = psg.tile([B, O_OUT], F32, tag="ps", bufs=8, name="psl")
                first = True
                for piece in range(4):
                    wlt = wlp.tile([128, 8000], MMDT, tag="wl_piece", name="wlt")
                    nc.sync.dma_start(wlt[:], wlp_ext.ap()[:, piece * 8000:(piece + 1) * 8000])
                    for yi in range(16):
                        for pr in range(5):
                            nc.tensor.matmul(
                                psl[:], d3p_v[:, pr, :, piece * 16 + yi],
                                wlt[:, (yi * 5 + pr) * O_OUT:(yi * 5 + pr + 1) * O_OUT],
                                start=first,
                                stop=(piece == 3 and yi == 15 and pr == 4))
                            first = False
                osb = lin['p'].tile([B, O_OUT], F32, name="osb")
                nc.vector.tensor_copy(osb[:], psl[:])
                nc.sync.dma_start(out_ext.ap(), osb[:])

        # ================= schedule =================
        l1_pair(0)
        l2_prep_start(0)
        l2_prep_slabs(0, 0, 3)
        l1_pair(1)
        l2_prep_slabs(0, 3, 6)
        l1_pair(2)
        l2_prep_slabs(0, 6, 9)
        l2_prep_fin(0)
        l1_pair(3)
        l1ctx.close()
        lin = lin_open()
        l2_prep(1)
        l2_mm(0)
        l2_prep(2)
        l2_mm(1)
        l2_prep(3)
        l2_mm(2)
        l3_prep(0)
        l2_mm(3)
        l3_mm(0)
        l3_prep(1)
        lin_prep(lin, 0)
        l3_mm(1)
        lin_prep(lin, 1)
        if (t := dbg_tap('h3', [64, 512])) is not None:
            nc.sync.dma_start(t.ap(), h3[:])
        lin_mm(lin)
        linctx.close()

    nc.compile()
    return nc

# ===================================================================== runner
from concourse.bass_utils import run_bass_kernel_spmd

_NC_CACHE = {}


def _get_nc():
    if 'nc' not in _NC_CACHE:
        _NC_CACHE['nc'] = build_nc(dbg=())
    return _NC_CACHE['nc']


def kernel(x, wb1, ws1, wb2, ws2, wb3, ws3, lb, lc):
    """Full-input entry point: x [64,3,64,64] f32 -> out [64,100] f32.
    Shards the batch over 8 NeuronCores (8 samples each), replicating weights."""
    x = np.ascontiguousarray(np.asarray(x, dtype=np.float32))
    w = fold_weights(np.asarray(wb1, np.float32), np.asarray(ws1, np.float32),
                     np.asarray(wb2, np.float32), np.asarray(ws2, np.float32),
                     np.asarray(wb3, np.float32), np.asarray(ws3, np.float32),
                     np.asarray(lb, np.float32), np.asarray(lc, np.float32))
    nc = _get_nc()
    in_maps = [{'x': x[i * B:(i + 1) * B], **w} for i in range(8)]
    res = run_bass_kernel_spmd(nc, in_maps, core_ids=list(range(8)))
    return np.concatenate([res.results[i]['out'] for i in range(8)], axis=0)


# revision 31
# speedup vs baseline: 1.0297x; 1.0265x over previous
"""CKANKANNet Trainium2 kernel (per-core SPMD program, B=8 samples/core).

Basis algorithm (d3-direct, weight-folded): with v = 2.5x+5.5 mapped to
integer knots and V = min(v, 11) via RY = relu(5.5-2.5x), the 3rd finite
difference of relu cubes saturates: d3_m(t) = t^3 - 3 relu(t-1)^3
+ 3 relu(t-2)^3 for t = clamp(V-m, 0, 3), constant 6 beyond. Each d3 slab
is THREE fused single-uop DVE ops straight from RY (clamps commuted into
the op bodies: u=max(min(RY+m-11,0),-3), m1=max(min(RY+m-10,0),-2),
m2=max(min(RY+m-9,0),-1); P1: 3*m1^3; P2: A - u^3; P3: A2 - 3*m2^3).
Since conv/linear are linear in the basis, the d4_j = d3_j - d3_{j+1}
difference is folded into the weights (w'_m = w_m - w_{m-1}, 9 channels):
no subtract passes, and L3/linear consume D3 tiles directly. The /6 is
folded into the weights too.

Convs: fp16 matmuls, channels on K partitions, 3x3 taps as accumulating
matmuls with edge-trimmed N ranges. L1 bakes ky taps into K via 3 shifted
channel-block copies (K=96, 30 live rows each). L2 K = 5 full 128-blocks
(4 m-pairs + [base|m8]). L3 K = 10 blocks (9 m + base). Image halves are
{0,1,4,5}/{2,3,6,7} at every layer so each layer's half-0 compute overlaps
the previous layer's half-1. Linear uses m-pair packed K=128 stationaries
(pr4 = [m8|base]).
"""
import sys
sys.path.insert(0, '/opt/trn_rl_repo')
from contextlib import ExitStack

import numpy as np
MM_NP = np.float16

import concourse.bass as bass
import concourse.tile as tile
from concourse import bacc, mybir
from concourse import dve_ops
from concourse.dve_spec import (Spec, Src0, Src1, sq, lower, minn, maxx,
                                _has_src1, C0, C1, C2, Zero, One)
from concourse.dve_uop import DveOpSpec

F32 = mybir.dt.float32
F16 = mybir.dt.float16
MMDT = mybir.dt.float16
AF = mybir.ActivationFunctionType
OP = mybir.AluOpType

B = 8
NB = 8
ND3 = 9
O_OUT = 100
IMGS = [[0, 1, 2, 3], [4, 5, 6, 7]]  # half -> image ids (all layers)


# ------------------------------------------------------------- custom DVE ops
def _register_dve_op(name, spec, subdim=False):
    if name in dve_ops._SUB_OPCODE_FOR_NAME:
        return next(op for op in dve_ops.OPS if op.name == name)
    row = max(dve_ops._SUB_OPCODE_FOR_NAME.values()) + 1
    assert row < 0x20
    ver = 'v3'
    tmp = DveOpSpec(name=name, opcode=row, uops=lower(spec, ver=ver),
                    rd1_en=_has_src1(spec))
    op = dve_ops.DveOp(name, spec, subdim, uops_sha={ver: tmp.sha(ver)})
    dve_ops.OPS.append(op)
    dve_ops.CUSTOM_DVE_SPECS[name] = spec
    dve_ops._SUB_OPCODE_FOR_NAME[name] = row
    return op


def _clamp_cube(in0, s0, s1):
    y = in0.astype(np.float32)
    m = np.maximum(np.minimum(y + s0, 0.0), s1)
    return m * m * m


def _d3p1_ref(in0, in1, s0, s1, imm2):
    return (imm2 * _clamp_cube(in0, s0, s1)).astype(np.float32)


def _d3p2_ref(in0, in1, s0, s1, imm2):
    return (in1.astype(np.float32) - _clamp_cube(in0, s0, s1)).astype(np.float32)


def _d3p3_ref(in0, in1, s0, s1, imm2):
    return (in1.astype(np.float32)
            - imm2 * _clamp_cube(in0, s0, s1)).astype(np.float32)


_c1 = maxx(minn(Src0 + C0, Zero), C1)
D3P1 = _register_dve_op('D3P1', Spec(body=C2 * (sq(_c1) * _c1),
                                     reference=_d3p1_ref))
_c2 = maxx(minn(Src0 + C0, Zero), C1)
D3P2 = _register_dve_op('D3P2', Spec(body=Src1 - sq(_c2) * _c2,
                                     reference=_d3p2_ref))
_c3 = maxx(minn(Src0 + C0, Zero), C1)
D3P3 = _register_dve_op('D3P3', Spec(body=Src1 - C2 * (sq(_c3) * _c3),
                                     reference=_d3p3_ref))


# ------------------------------------------------------------- weight folding
def silu_np(x):
    return x / (1.0 + np.exp(-x))


def d3_fold(blk):
    """blk: [NB(j), ...]. Returns [9(m), ...] with w'_m = w_m - w_{m-1}."""
    out = np.zeros((ND3,) + blk.shape[1:], blk.dtype)
    out[0:NB] = blk
    out[1:ND3] -= blk
    return out


def fold_weights(wb1, ws1, wb2, ws2, wb3, ws3, lb, lc):
    out = {}
    W1 = np.zeros((96, 3 * 64), np.float32)
    for kyi in range(3):
        for kxi in range(3):
            W1[kyi * 32 + 0:kyi * 32 + 3, kxi * 64:(kxi + 1) * 64] = wb1[:, :, kyi, kxi].T
            blk = np.transpose(ws1[:, :, kyi, kxi].reshape(64, 3, NB), (2, 1, 0)) / 6.0
            W1[kyi * 32 + 3:kyi * 32 + 30, kxi * 64:(kxi + 1) * 64] = \
                d3_fold(blk).reshape(27, 64)
    out['w1'] = W1.astype(MM_NP)

    W2 = np.zeros((640, 9 * 128), np.float32)
    for kyi in range(3):
        for kxi in range(3):
            t = kyi * 3 + kxi
            blk = np.transpose(ws2[:, :, kyi, kxi].reshape(128, 64, NB), (2, 1, 0)) / 6.0
            blk9 = d3_fold(blk).reshape(576, 128)
            W2[0:512, t * 128:(t + 1) * 128] = blk9[0:512]
            W2[512:576, t * 128:(t + 1) * 128] = wb2[:, :, kyi, kxi].T
            W2[576:640, t * 128:(t + 1) * 128] = blk9[512:576]
    out['w2'] = W2.astype(MM_NP)

    W3 = np.zeros((1280, 9 * 64), np.float32)
    for kyi in range(3):
        for kxi in range(3):
            t = kyi * 3 + kxi
            blk = np.transpose(ws3[:, :, kyi, kxi].reshape(64, 128, NB), (2, 1, 0)) / 6.0
            W3[0:1152, t * 64:(t + 1) * 64] = d3_fold(blk).reshape(1152, 64)
            W3[1152:1280, t * 64:(t + 1) * 64] = wb3[:, :, kyi, kxi].T
    out['w3'] = W3.astype(MM_NP)

    # linear: m-pair packed stationaries (K=128); pr4 = [m8 | base]
    lc9 = d3_fold(np.transpose(lc.reshape(O_OUT, 64, 64, NB), (3, 0, 1, 2)) / 6.0)
    lb_r = lb.reshape(O_OUT, 64, 64)
    WLP = np.zeros((128, 4, 16, 5, O_OUT), np.float32)
    for p in range(4):
        for yi in range(16):
            yx = p * 16 + yi
            for pr in range(4):
                WLP[0:64, p, yi, pr, :] = lc9[2 * pr, :, :, yx].T
                WLP[64:128, p, yi, pr, :] = lc9[2 * pr + 1, :, :, yx].T
            WLP[0:64, p, yi, 4, :] = lc9[8, :, :, yx].T
            WLP[64:128, p, yi, 4, :] = lb_r[:, :, yx].T
    out['wlp'] = WLP.reshape(128, 4 * 16 * 5 * O_OUT).astype(MM_NP)
    return out


# ------------------------------------------------------------- basis emission
def emit_d3_start(nc, tpool, d3pool, src_ap, P, E, bias_tiles, name=""):
    """Scalar RY pass + D3 tile allocation; slabs emitted via emit_d3_slabs."""
    RY = tpool.tile([P, E], F32, tag="ry", name=f"RY{name}")
    nc.scalar.activation(RY[:], src_ap, AF.Relu, bias=bias_tiles['b55'][0:P, :],
                         scale=-2.5)
    D3 = d3pool.tile([P, ND3 * E], F16, tag="d3", name=f"D3{name}")
    return RY, D3


def emit_d3_slabs(nc, tpool, RY, D3, P, E, m0, m1, name=""):
    """Emit d3 slabs m0..m1: three fused single-uop DVE ops per slab."""
    for m in range(m0, m1):
        A = tpool.tile([P, E], F32, tag="a", name=f"A{name}_{m}")
        nc.vector._custom_dve(D3P1, out=A[:], in0=RY[:], s0=float(m - 10),
                              s1=-2.0, imm2=3.0)
        A2 = tpool.tile([P, E], F32, tag="a2", name=f"A2{name}_{m}")
        nc.vector._custom_dve(D3P2, out=A2[:], in0=RY[:], in1=A[:],
                              s0=float(m - 11), s1=-3.0)
        nc.vector._custom_dve(D3P3, out=D3[:, m * E:(m + 1) * E], in0=RY[:],
                              in1=A2[:], s0=float(m - 9), s1=-1.0, imm2=3.0)


def emit_d3(nc, tpool, d3pool, src_ap, P, E, bias_tiles, name=""):
    RY, D3 = emit_d3_start(nc, tpool, d3pool, src_ap, P, E, bias_tiles, name)
    emit_d3_slabs(nc, tpool, RY, D3, P, E, 0, ND3, name)
    return D3


def maxpool_from_psum(nc, psum_ap, n_bh, W_half, out_ap):
    pv = psum_ap.rearrange("p (hp r2 wp c2) -> p hp wp r2 c2",
                           hp=n_bh, r2=2, wp=W_half, c2=2)
    nc.vector.tensor_reduce(out_ap.rearrange("p (hp wp) -> p hp wp", wp=W_half),
                            pv, mybir.AxisListType.XY, OP.max, opt_input=False)


# ----------------------------------------------------------------- the kernel
def build_nc(dbg=()):
    nc = bacc.Bacc("TRN2", target_bir_lowering=False, debug=False, num_devices=8)
    x_ext = nc.declare_dram_parameter("x", [B, 3, 64, 64], F32, isOutput=False)
    w1_ext = nc.declare_dram_parameter("w1", [96, 192], MMDT, isOutput=False)
    w2_ext = nc.declare_dram_parameter("w2", [640, 1152], MMDT, isOutput=False)
    w3_ext = nc.declare_dram_parameter("w3", [1280, 576], MMDT, isOutput=False)
    wlp_ext = nc.declare_dram_parameter("wlp", [128, 32000], MMDT, isOutput=False)
    out_ext = nc.declare_dram_parameter("out", [B, O_OUT], F32, isOutput=True)

    dbg_exts = {}

    def dbg_tap(name, shape, dt=F32):
        if name in dbg:
            dbg_exts[name] = nc.declare_dram_parameter(f"dbg_{name}", shape, dt, isOutput=True)
            return dbg_exts[name]
        return None

    with tile.TileContext(nc) as tc, ExitStack() as ctx:
        persist = ctx.enter_context(tc.tile_pool(name="persist", bufs=1))
        wpool = ctx.enter_context(tc.tile_pool(name="wpool", bufs=1))
        tpool = ctx.enter_context(tc.tile_pool(name="tpool", bufs=2))
        d3pool = ctx.enter_context(tc.tile_pool(name="d3pool", bufs=2))
        l23w = ctx.enter_context(tc.tile_pool(name="l23w", bufs=1))
        consp = ctx.enter_context(tc.tile_pool(name="cons", bufs=1))
        psg = ctx.enter_context(tc.tile_pool(name="psg", bufs=1, space="PSUM"))

        bias_tiles = {}
        bt_55 = wpool.tile([128, 1], F32, tag="bias_55", name="bias55")
        nc.gpsimd.memset(bt_55[:], 5.5)
        bias_tiles['b55'] = bt_55
        zt = wpool.tile([30, 64], MMDT)
        nc.gpsimd.memset(zt[:], 0.0)

        h1h = [persist.tile([64, 4096], F16, tag=f"h1_{i}", name=f"h1_{i}")
               for i in range(2)]
        h2h = [persist.tile([128, 1024], F32, tag=f"h2_{i}", name=f"h2_{i}")
               for i in range(2)]
        h3 = persist.tile([64, 512], F32)

        w2sb = [l23w.tile([128, 1152], MMDT, tag=f"w2_{i}", name=f"w2sb{i}")
                for i in range(5)]
        w3sb = [l23w.tile([128, 576], MMDT, tag=f"w3_{i}", name=f"w3sb{i}")
                for i in range(10)]
        h1s = l23w.tile([128, 4096], F16)

        T2w = {}  # wave -> list of 5 T2 tiles

        # ================= L1 (prologue) =================
        l1ctx = ExitStack()
        l1p = l1ctx.enter_context(tc.tile_pool(name="l1pool", bufs=1))
        X1 = l1p.tile([128, 768], F32)
        for c in range(3):
            nc.sync.dma_start(
                X1[:, c * 256:(c + 1) * 256],
                x_ext.ap()[:, c, :, :].rearrange("b (g hh) w -> b g (hh w)", g=16))
        w1sb = wpool.tile([96, 192], MMDT)
        nc.sync.dma_start(w1sb[:], w1_ext.ap())
        # preload L2/L3/linear-stationary weights during the prologue
        for i in range(5):
            nc.sync.dma_start(w2sb[i][:], w2_ext.ap()[i * 128:(i + 1) * 128, :])
        for i in range(10):
            nc.sync.dma_start(w3sb[i][:], w3_ext.ap()[i * 128:(i + 1) * 128, :])

        sl1 = l1p.tile([128, 768], MMDT)
        nc.scalar.activation(sl1[:], X1[:], AF.Silu)
        D3_1 = emit_d3(nc, tpool, d3pool, X1[:], 128, 768, bias_tiles,
                       name="l1")

        # dump channels to DRAM (ch-major), then read back partition-parallel
        # into the three ky-shifted 30-row blocks. Pair-0 reads are split and
        # interleaved with the dumps so its bun assembles as slabs land.
        l1ch = nc.dram_tensor("l1ch", [30, B * 4096], MMDT)
        for c in range(3):
            nc.sync.dma_start(
                l1ch.ap()[c, :].rearrange("(bg e) -> bg e", e=256),
                sl1[:, c * 256:(c + 1) * 256])
        for m in range(4):
            nc.sync.dma_start(
                l1ch.ap()[3 + m * 3:3 + m * 3 + 3, :]
                    .rearrange("c (bg e) -> bg c e", e=256),
                D3_1[:, m * 768:(m + 1) * 768]
                    .rearrange("p (c e) -> p c e", e=256))
        Bun0 = l1p.tile([96, 64 + 2 * 4096 + 64], MMDT, tag="bun1",
                        bufs=2, name="Bun1_0")
        for kyi in range(3):
            base = 64 + (1 - kyi) * 64
            nc.sync.dma_start(Bun0[kyi * 32:kyi * 32 + 15, base:base + 8192],
                              l1ch.ap()[0:15, 0:8192])
            nc.sync.dma_start(
                Bun0[kyi * 32 + 30:kyi * 32 + 32, 0:8320],
                l1ch.ap()[0:2, 0:8320])
        for m in range(4, ND3):
            nc.sync.dma_start(
                l1ch.ap()[3 + m * 3:3 + m * 3 + 3, :]
                    .rearrange("c (bg e) -> bg c e", e=256),
                D3_1[:, m * 768:(m + 1) * 768]
                    .rearrange("p (c e) -> p c e", e=256))
        for kyi in range(3):
            base = 64 + (1 - kyi) * 64
            nc.sync.dma_start(Bun0[kyi * 32 + 15:kyi * 32 + 30, base:base + 8192],
                              l1ch.ap()[15:30, 0:8192])

        def l1_pair(pi):
            # pair pi covers images (2pi, 2pi+1) == L2 wave pi's image pair
            b0 = 2 * pi
            if pi == 0:
                Bun = Bun0
            else:
                Bun = l1p.tile([96, 64 + 2 * 4096 + 64], MMDT, tag="bun1",
                               bufs=2, name=f"Bun1_{pi}")
                for kyi in range(3):
                    base = 64 + (1 - kyi) * 64
                    nc.sync.dma_start(
                        Bun[kyi * 32:kyi * 32 + 30, base:base + 8192],
                        l1ch.ap()[:, b0 * 4096:(b0 + 2) * 4096])
                    # pad rows 30:32: duplicate finite data (zero weights)
                    nc.sync.dma_start(
                        Bun[kyi * 32 + 30:kyi * 32 + 32, 0:8320],
                        l1ch.ap()[0:2, 0:8320])
            # boundary rows: ky=0 block box-row 0; ky=2 block box-row 63
            for bi in range(2):
                nc.sync.dma_start(
                    Bun[0:30, 64 + bi * 4096: 64 + bi * 4096 + 64], zt[:])
                nc.sync.dma_start(
                    Bun[64:94, 64 + bi * 4096 + 63 * 64: 64 + bi * 4096 + 64 * 64],
                    zt[:])

            bun_v = Bun[:, 64:64 + 2 * 4096].rearrange(
                "p (b r w) -> p b r w", b=2, w=64)
            bh, sp = pi // 2, pi % 2
            chunks = [(bi, hb) for bi in range(2) for hb in range(8)]
            for g in range(0, len(chunks), 8):
                grp = chunks[g:g + 8]
                pss = [psg.tile([64, 512], F32, tag="ps", bufs=8,
                                name=f"ps1_{pi}_{g}_{i}")
                       for i in range(len(grp))]
                for ti, kxi in enumerate([1, 0, 2]):
                    for ci, (bi, hb) in enumerate(grp):
                        ps = pss[ci]
                        if kxi == 0:
                            mv = bun_v[:, bi, hb * 8:hb * 8 + 8, 0:63]
                            ov = ps[:].rearrange("p (r w) -> p r w", w=64)[:, :, 1:64]
                        elif kxi == 1:
                            mv = bun_v[:, bi, hb * 8:hb * 8 + 8, :]
                            ov = ps[:]
                        else:
                            mv = bun_v[:, bi, hb * 8:hb * 8 + 8, 1:64]
                            ov = ps[:].rearrange("p (r w) -> p r w", w=64)[:, :, 0:63]
                        nc.tensor.matmul(ov, w1sb[:, kxi * 64:(kxi + 1) * 64], mv,
                                         start=(ti == 0), stop=(ti == 2))
                for ci, (bi, hb) in enumerate(grp):
                    slot = 2 * sp + bi
                    maxpool_from_psum(nc, pss[ci][:], 4, 32,
                                      h1h[bh][:, slot * 1024 + hb * 128:
                                              slot * 1024 + (hb + 1) * 128])

        # ================= L2 / L3 bodies =================
        W0S = {}

        def l2_prep_start(wv):
            bh, par = wv // 2, wv % 2
            col0 = bh * 2048 + par * 1024
            if par == 0:
                # issue on gpsimd (SWDGE) to keep the sync queue free for buns
                nc.gpsimd.dma_start(h1s[0:64, bh * 2048:(bh + 1) * 2048],
                                    h1h[bh][:, 0:2048])
                nc.gpsimd.dma_start(h1s[64:128, bh * 2048:(bh + 1) * 2048],
                                    h1h[bh][:, 2048:4096])
            T2 = [consp.tile([128, 2048], MMDT, tag=f"t2_{i}_{wv % 2}",
                             name=f"T2_{i}_{wv % 2}") for i in range(5)]
            T2w[wv] = T2
            RY, D3t = emit_d3_start(nc, tpool, d3pool, h1s[:, col0:col0 + 1024],
                                    128, 1024, bias_tiles, name=f"l2_{wv}")
            W0S[wv] = (RY, D3t)

        def l2_prep_slabs(wv, m0, m1):
            E = 1024
            RY, D3t = W0S[wv]
            T2 = T2w[wv]
            emit_d3_slabs(nc, tpool, RY, D3t, 128, E, m0, m1, name=f"l2_{wv}")
            for m in range(m0, m1):
                for ph in range(2):
                    if m < NB:
                        dst = T2[m // 2][(m % 2) * 64:(m % 2) * 64 + 64,
                                         ph * 1024:(ph + 1) * 1024]
                    else:
                        dst = T2[4][64:128, ph * 1024:(ph + 1) * 1024]
                    nc.scalar.dma_start(dst,
                                        D3t[ph * 64:(ph + 1) * 64,
                                            m * E:(m + 1) * E])

        def l2_prep_fin(wv):
            bh, par = wv // 2, wv % 2
            T2 = T2w[wv]
            del W0S[wv]
            # base silu -> T2[4] rows 0:64 (partition-aligned with h1h)
            for ii in range(2):
                slot = par + 2 * ii
                nc.scalar.activation(T2[4][0:64, ii * 1024:(ii + 1) * 1024],
                                     h1h[bh][:, slot * 1024:(slot + 1) * 1024],
                                     AF.Silu)

        def l2_prep(wv):
            l2_prep_start(wv)
            l2_prep_slabs(wv, 0, ND3)
            l2_prep_fin(wv)

        def l2_mm(wv):
            bh, par = wv // 2, wv % 2
            T2 = T2w.pop(wv)
            t2v = [T2[i][:].rearrange("p (b h w) -> p b h w", b=2, w=32)
                   for i in range(5)]
            taps = [(0, 1, 1)] + [(kt, kyi, kxi) for kt in range(5)
                                  for kyi in range(3) for kxi in range(3)
                                  if (kt, kyi, kxi) != (0, 1, 1)]
            n_taps = len(taps)
            chunks = [(ii, half) for ii in range(2) for half in range(2)]
            pss = [psg.tile([128, 512], F32, tag="ps", bufs=8,
                            name=f"ps2_{wv}_{i}") for i in range(4)]
            for tapi, (kt, kyi, kxi) in enumerate(taps):
                for ci, (ii, half) in enumerate(chunks):
                    h0 = half * 16
                    ps = pss[ci]
                    r_lo = max(0, 1 - kyi - h0)
                    r_hi = min(16, 33 - h0 - kyi)
                    w_lo = 1 if kxi == 0 else 0
                    w_hi = 31 if kxi == 2 else 32
                    in_row = h0 + r_lo + kyi - 1
                    in_col = w_lo + kxi - 1
                    mv = t2v[kt][:, ii, in_row:in_row + (r_hi - r_lo),
                                 in_col:in_col + (w_hi - w_lo)]
                    ov = ps[:].rearrange("p (r w) -> p r w", w=32)[
                        :, r_lo:r_hi, w_lo:w_hi]
                    nc.tensor.matmul(
                        ov,
                        w2sb[kt][:, (kyi * 3 + kxi) * 128:
                                 (kyi * 3 + kxi + 1) * 128],
                        mv, start=(tapi == 0),
                        stop=(tapi == n_taps - 1))
            for ci, (ii, half) in enumerate(chunks):
                slot = par + 2 * ii
                maxpool_from_psum(nc, pss[ci][:], 8, 16,
                                  h2h[bh][:, slot * 256 + half * 128:
                                          slot * 256 + (half + 1) * 128])

        D3l3 = {}

        def l3_prep(bh):
            E = 1024
            D3t = emit_d3(nc, tpool, d3pool, h2h[bh][:], 128, E,
                          bias_tiles, name="l3")
            Ts3 = consp.tile([128, 1024], MMDT, tag=f"ts3_{bh}", name=f"Ts3_{bh}")
            for p in range(2):
                nc.scalar.activation(Ts3[:, p * 512:(p + 1) * 512],
                                     h2h[bh][:, p * 512:(p + 1) * 512], AF.Silu)
            D3l3[bh] = (D3t, Ts3)

        def l3_mm(bh):
            imgs = IMGS[bh]
            E = 1024
            D3t, Ts3 = D3l3.pop(bh)
            t3v = [D3t[:, m * E:(m + 1) * E].rearrange(
                       "p (b h w) -> p b h w", b=4, w=16)
                   for m in range(ND3)]
            ts3v = Ts3[:].rearrange("p (b h w) -> p b h w", b=4, w=16)
            taps = [(0, 1, 1)] + [(kt, kyi, kxi) for kt in range(10)
                                  for kyi in range(3) for kxi in range(3)
                                  if (kt, kyi, kxi) != (0, 1, 1)]
            n_taps = len(taps)
            pss = [psg.tile([64, 512], F32, tag="ps", bufs=8,
                            name=f"ps3_{bh}_{i}") for i in range(2)]
            for tapi, (kt, kyi, kxi) in enumerate(taps):
                for ckc in range(2):
                    b0c = ckc * 2
                    ps = pss[ckc]
                    r_lo = max(0, 1 - kyi)
                    r_hi = min(16, 17 - kyi)
                    w_lo = 1 if kxi == 0 else 0
                    w_hi = 15 if kxi == 2 else 16
                    src_ = t3v[kt] if kt < ND3 else ts3v
                    mv = src_[:, b0c:b0c + 2, r_lo + kyi - 1:r_hi + kyi - 1,
                              w_lo + kxi - 1:w_lo + kxi - 1 + (w_hi - w_lo)]
                    ov = ps[:].rearrange("p (b r w) -> p b r w", b=2, w=16)[
                        :, :, r_lo:r_hi, w_lo:w_hi]
                    nc.tensor.matmul(
                        ov,
                        w3sb[kt][:, (kyi * 3 + kxi) * 64:
                                 (kyi * 3 + kxi + 1) * 64],
                        mv, start=(tapi == 0),
                        stop=(tapi == n_taps - 1))
            for ckc in range(2):
                b0i = imgs[2 * ckc]
                maxpool_from_psum(nc, pss[ckc][:], 16, 8,
                                  h3[:, b0i * 64:(b0i + 2) * 64])

        # ================= Linear bodies =================
        linctx = ExitStack()

        def lin_open():
            # l1pool space is free by now; reuse it for the linear tiles
            lin = {}
            lin['p'] = linctx.enter_context(tc.tile_pool(name="linp", bufs=1))
            lin['sl3'] = lin['p'].tile([64, 512], MMDT, name="sl3")
            lin['D3P'] = lin['p'].tile([128, 5 * 512], MMDT, name="D3P")
            return lin

        def lin_prep(lin, half):
            # emit basis for images [4*half, 4*half+4) (contiguous h3 cols)
            sl3 = lin['sl3']
            nc.scalar.activation(sl3[:, half * 256:(half + 1) * 256],
                                 h3[:, half * 256:(half + 1) * 256], AF.Silu)
            D3L = emit_d3(nc, tpool, d3pool, h3[:, half * 256:(half + 1) * 256],
                          64, 256, bias_tiles, name=f"lin{half}")
            # contiguous moves: D3P cols (pr, b, yx) put each half's 4 images
            # in one 256-col block per pr; rows 0:64 even m, 64:128 odd m.
            D3Pt = lin['D3P']
            for pr in range(4):
                c0 = pr * 512 + half * 256
                iss = nc.sync if pr % 2 == 0 else nc.scalar
                iss.dma_start(D3Pt[0:64, c0:c0 + 256],
                              D3L[:, (2 * pr) * 256:(2 * pr + 1) * 256])
                iss.dma_start(D3Pt[64:128, c0:c0 + 256],
                              D3L[:, (2 * pr + 1) * 256:(2 * pr + 2) * 256])
            c0 = 4 * 512 + half * 256
            nc.sync.dma_start(D3Pt[0:64, c0:c0 + 256],
                              D3L[:, 8 * 256:9 * 256])
            nc.scalar.dma_start(D3Pt[64:128, c0:c0 + 256],
                                sl3[:, half * 256:(half + 1) * 256])

        def lin_mm(lin):
            d3p_v = lin['D3P'][:].rearrange("p (pr b yx) -> p pr b yx", pr=5, b=B)
            with tc.tile_pool(name="wlpool", bufs=2) as wlp:
                psl = psg.tile([B, O_OUT], F32, tag="ps", bufs=8, name="psl")
                first = True
                for piece in range(4):
                    wlt = wlp.tile([128, 8000], MMDT, tag="wl_piece", name="wlt")
                    nc.sync.dma_start(wlt[:], wlp_ext.ap()[:, piece * 8000:(piece + 1) * 8000])
                    for yi in range(16):
                        for pr in range(5):
                            nc.tensor.matmul(
                                psl[:], d3p_v[:, pr, :, piece * 16 + yi],
                                wlt[:, (yi * 5 + pr) * O_OUT:(yi * 5 + pr + 1) * O_OUT],
                                start=first,
                                stop=(piece == 3 and yi == 15 and pr == 4))
                            first = False
                osb = lin['p'].tile([B, O_OUT], F32, name="osb")
                nc.vector.tensor_copy(osb[:], psl[:])
                nc.sync.dma_start(out_ext.ap(), osb[:])

        # ================= schedule =================
        l1_pair(0)
        l1_pair(1)
        l2_prep_start(0)
        l2_prep_slabs(0, 0, 3)
        l1_pair(2)
        l2_prep_slabs(0, 3, 6)
        l1_pair(3)
        l2_prep_slabs(0, 6, 9)
        l2_prep_fin(0)
        l1ctx.close()
        lin = lin_open()
        l2_prep(1)
        l2_mm(0)
        l2_prep(2)
        l2_mm(1)
        l2_prep(3)
        l2_mm(2)
        l3_prep(0)
        l2_mm(3)
        l3_mm(0)
        l3_prep(1)
        lin_prep(lin, 0)
        l3_mm(1)
        lin_prep(lin, 1)
        if (t := dbg_tap('h3', [64, 512])) is not None:
            nc.sync.dma_start(t.ap(), h3[:])
        lin_mm(lin)
        linctx.close()

    nc.compile()
    return nc

# ===================================================================== runner
from concourse.bass_utils import run_bass_kernel_spmd

_NC_CACHE = {}


def _get_nc():
    if 'nc' not in _NC_CACHE:
        _NC_CACHE['nc'] = build_nc(dbg=())
    return _NC_CACHE['nc']


def kernel(x, wb1, ws1, wb2, ws2, wb3, ws3, lb, lc):
    """Full-input entry point: x [64,3,64,64] f32 -> out [64,100] f32.
    Shards the batch over 8 NeuronCores (8 samples each), replicating weights."""
    x = np.ascontiguousarray(np.asarray(x, dtype=np.float32))
    w = fold_weights(np.asarray(wb1, np.float32), np.asarray(ws1, np.float32),
                     np.asarray(wb2, np.float32), np.asarray(ws2, np.float32),
                     np.asarray(wb3, np.float32), np.asarray(ws3, np.float32),
                     np.asarray(lb, np.float32), np.asarray(lc, np.float32))
    nc = _get_nc()
    in_maps = [{'x': x[i * B:(i + 1) * B], **w} for i in range(8)]
    res = run_bass_kernel_spmd(nc, in_maps, core_ids=list(range(8)))
    return np.concatenate([res.results[i]['out'] for i in range(8)], axis=0)
